# revision 11
# baseline (speedup 1.0000x reference)
"""GATv2 layer (4 heads x 64ch, N=50000, E=800000) on 8 Trainium2 NeuronCores.

Strategy (dst-sharded, SPMD single NEFF):
- Host: add self-loops, shard edges by dst node (6250 dst/core), sort by dst,
  group into 128-dst windows, pack into 128-edge subtiles (padded to a common
  per-window subtile count across cores so one program serves all 8 cores).
- Phase 1 (device, replicated): xl = x @ W_l + b_l for ALL nodes -> fp16 table
  in DRAM (each core computes its own copy; avoids cross-core collectives);
  xr = x @ W_r + b_r for the core's own dst shard.
- Phase 2 (device): per window, dma_gather xl[src] / xr[dst] rows (512B fp16),
  m = xl+xr (DVE), leaky-relu via Prelu(0.2) (ACT), u = p*att (DVE),
  per-head tree reduction -> logits, shifted exp (fp16-safe), onehot(dst slot)
  built by is_equal vs iota, PE matmuls accumulate sum(ex * xl[src]) and
  sum(ex) per 128-dst window in PSUM.  Normalization is factored out of the
  per-edge path: out = num/den at window end, then +bias, LayerNorm, ReLU.
"""
import sys
import os
import numpy as np

sys.path.insert(0, '/opt/trn_rl_repo')

N = 50000
IN_C = 64
OUT_C = 64
HEADS = 4
HC = HEADS * OUT_C          # 256
E = 800000
NEG_SLOPE = 0.2
LN_EPS = 1e-5
NCORES = 8
SH = N // NCORES            # 6250 dst nodes per core
WIN = 128
NW = (SH + WIN - 1) // WIN  # 49 windows (last window 106 dst)
HI_BASE = 32768             # int16 gather index split point
SLAB = 2048                 # phase-1 column slab


def _pack_idx(v):
    """Flat idx vector (len % 16 == 0) -> [128, len/16] int16 (idx k at
    (k%16, k//16), replicated across the 8 gpsimd cores' 16-partition stripes)."""
    a = v.reshape(-1, 16).T
    return np.tile(a, (8, 1)).astype(np.int16)


def _prep(x, edge_index, W_l, b_l, W_r, b_r, att, bias, ln_gamma, ln_beta):
    src = np.asarray(edge_index[0], dtype=np.int64)
    dst = np.asarray(edge_index[1], dtype=np.int64)
    loops = np.arange(N, dtype=np.int64)
    src = np.concatenate([src, loops])
    dst = np.concatenate([dst, loops])

    core = dst // SH
    dstl = dst % SH
    win = dstl // WIN
    half = (src >= HI_BASE).astype(np.int64)

    order = np.lexsort((dstl, half, win, core))
    src = src[order]; dst = dst[order]
    core = core[order]; dstl = dstl[order]; win = win[order]; half = half[order]

    # counts[core, win, half]
    key = (core * NW + win) * 2 + half
    cnt = np.bincount(key, minlength=NCORES * NW * 2).reshape(NCORES, NW, 2)
    subs = -(-cnt // 128)                      # ceil
    T_lh = subs.max(axis=0)                    # [NW, 2] shared subtile counts
    T_low = T_lh[:, 0]; T_high = T_lh[:, 1]
    T_w = T_low + T_high
    S_total = int(T_w.sum())
    off_w = np.zeros(NW, dtype=np.int64)
    off_w[1:] = np.cumsum(T_w)[:-1]

    # slot base for each (core, win, half) group in the flat padded stream
    grp_off = np.zeros((NCORES, NW, 2), dtype=np.int64)
    for c in range(NCORES):
        for w in range(NW):
            grp_off[c, w, 0] = off_w[w] * 128
            grp_off[c, w, 1] = (off_w[w] + T_low[w]) * 128

    NSLOT = S_total * 128
    # within-group rank of each edge (edges are sorted by (core,win,half))
    grp_start = np.zeros(NCORES * NW * 2, dtype=np.int64)
    grp_start[1:] = np.cumsum(cnt.reshape(-1))[:-1]
    rank = np.arange(len(src)) - grp_start[key]
    slot = grp_off[core, win, half] + rank

    xl_idx = np.zeros((NCORES, NSLOT), dtype=np.int16)
    xr_idx = np.zeros((NCORES, NSLOT), dtype=np.int16)
    ds_val = np.full((NCORES, NSLOT), 254.0, dtype=np.float16)
    xl_val = np.where(half == 1, src - HI_BASE, src).astype(np.int16)
    for c in range(NCORES):
        m = core == c
        xl_idx[c, slot[m]] = xl_val[m]
        xr_idx[c, slot[m]] = dstl[m].astype(np.int16)
        ds_val[c, slot[m]] = (dstl[m] - win[m] * WIN).astype(np.float16)

    # per-head exp shift from a sample of edges (keeps exp in fp16 range)
    rs = np.random.RandomState(1234)
    samp = rs.randint(0, len(src), min(32768, len(src)))
    xs = np.asarray(x, dtype=np.float32)
    Wl = np.asarray(W_l, np.float32); Wr = np.asarray(W_r, np.float32)
    bl = np.asarray(b_l, np.float32); br = np.asarray(b_r, np.float32)
    attf = np.asarray(att, np.float32).reshape(HEADS, OUT_C)
    ms = (xs[src[samp]] @ Wl + bl) + (xs[dst[samp]] @ Wr + br)
    ls = np.where(ms > 0, ms, NEG_SLOPE * ms).reshape(-1, HEADS, OUT_C)
    lg = np.einsum('ehc,hc->eh', ls, attf)
    c_shift = (lg.max(axis=0) - 6.0).astype(np.float32)

    # device-side constant tensors
    xT = np.ascontiguousarray(xs.T).astype(np.float16)              # [64, N]
    W_aug = np.zeros((65, 2 * HC), dtype=np.float16)
    W_aug[:64, :HC] = Wl; W_aug[64, :HC] = bl
    W_aug[:64, HC:] = Wr; W_aug[64, HC:] = br
    att_b = np.broadcast_to(attf.reshape(-1).astype(np.float16), (128, HC)).copy()
    iota = np.broadcast_to(np.arange(128, dtype=np.float16), (128, 128)).copy()
    c_b = np.broadcast_to(c_shift, (128, HEADS)).copy()

    biasf = np.asarray(bias, np.float32)
    gam = np.asarray(ln_gamma, np.float32)
    bet = np.asarray(ln_beta, np.float32)
    use_bias = bool(np.any(biasf != 0.0))
    use_gam = bool(np.any(gam != 1.0))
    use_bet = bool(np.any(bet != 0.0))

    per_core = []
    for c in range(NCORES):
        per_core.append({
            "xT": xT,
            "xTs": np.ascontiguousarray(xT[:, c * SH:(c + 1) * SH]),
            "W": W_aug,
            "attb": att_b,
            "iota": iota,
            "cshift": c_b.astype(np.float16),
            "xli": _pack_idx(xl_idx[c]),
            "xri": _pack_idx(xr_idx[c]),
            "dslot": np.ascontiguousarray(ds_val[c].reshape(-1, 128).T).astype(np.float16),
            "biasb": np.broadcast_to(biasf, (128, HC)).astype(np.float32).copy(),
            "gamb": np.broadcast_to(gam, (128, HC)).astype(np.float32).copy(),
            "betb": np.broadcast_to(bet, (128, HC)).astype(np.float32).copy(),
        })
    struct = {
        "T_low": T_low.tolist(), "T_high": T_high.tolist(),
        "off_w": off_w.tolist(), "S_total": S_total,
        "use_bias": use_bias, "use_gam": use_gam, "use_bet": use_bet,
    }
    return per_core, struct


def _build(struct):
    import concourse.bacc as bacc
    import concourse.mybir as mybir
    import concourse.tile as tile

    F16 = mybir.dt.float16
    F32 = mybir.dt.float32
    I16 = mybir.dt.int16
    AT = mybir.AluOpType
    AF = mybir.ActivationFunctionType

    T_low = struct["T_low"]; T_high = struct["T_high"]
    off_w = struct["off_w"]; S = struct["S_total"]

    nc = bacc.Bacc("TRN2", num_devices=NCORES, num_swdge_queues=4)

    xT_d = nc.dram_tensor("xT", [64, N], F16, kind="ExternalInput")
    xTs_d = nc.dram_tensor("xTs", [64, SH], F16, kind="ExternalInput")
    W_d = nc.dram_tensor("W", [65, 2 * HC], F16, kind="ExternalInput")
    attb_d = nc.dram_tensor("attb", [128, HC], F16, kind="ExternalInput")
    iota_d = nc.dram_tensor("iota", [128, 128], F16, kind="ExternalInput")
    csh_d = nc.dram_tensor("cshift", [128, HEADS], F16, kind="ExternalInput")
    xli_d = nc.dram_tensor("xli", [128, 8 * S], I16, kind="ExternalInput")
    xri_d = nc.dram_tensor("xri", [128, 8 * S], I16, kind="ExternalInput")
    ds_d = nc.dram_tensor("dslot", [128, S], F16, kind="ExternalInput")
    biasb_d = nc.dram_tensor("biasb", [128, HC], F32, kind="ExternalInput")
    gamb_d = nc.dram_tensor("gamb", [128, HC], F32, kind="ExternalInput")
    betb_d = nc.dram_tensor("betb", [128, HC], F32, kind="ExternalInput")
    y_d = nc.dram_tensor("y", [SH, HC], F32, kind="ExternalOutput")

    xl_lo = nc.dram_tensor("xl_lo", [HI_BASE, HC], F16, kind="Internal")
    xl_hi = nc.dram_tensor("xl_hi", [N - HI_BASE, HC], F16, kind="Internal")
    xr_dram = nc.dram_tensor("xr_tab", [SH, HC], F16, kind="Internal")

    with tile.TileContext(nc) as tc:
        with tc.tile_pool(name="const", bufs=1) as cp, \
             tc.tile_pool(name="slab", bufs=2) as slp, \
             tc.tile_pool(name="ev", bufs=3) as evp, \
             tc.tile_pool(name="g", bufs=4) as gp, \
             tc.tile_pool(name="gm", bufs=3) as gm, \
             tc.tile_pool(name="wk", bufs=2) as wk, \
             tc.tile_pool(name="wc", bufs=3) as wc, \
             tc.tile_pool(name="ln", bufs=3) as lnp, \
             tc.tile_pool(name="zr", bufs=12) as zp, \
             tc.tile_pool(name="p1", bufs=4, space="PSUM") as p1, \
             tc.tile_pool(name="pA", bufs=3, space="PSUM") as pA:

            # ---- constants ----
            W_t = cp.tile([65, 2 * HC], F16)
            nc.sync.dma_start(W_t[:], W_d[:])
            att_t = cp.tile([128, HC], F16)
            nc.sync.dma_start(att_t[:], attb_d[:])
            iota_t = cp.tile([128, 128], F16)
            nc.sync.dma_start(iota_t[:], iota_d[:])
            csh_t = cp.tile([128, HEADS], F16)
            nc.sync.dma_start(csh_t[:], csh_d[:])
            xli_t = cp.tile([128, 8 * S], I16)
            nc.sync.dma_start(xli_t[:], xli_d[:])
            xri_t = cp.tile([128, 8 * S], I16)
            nc.sync.dma_start(xri_t[:], xri_d[:])
            ds_t = cp.tile([128, S], F16)
            nc.sync.dma_start(ds_t[:], ds_d[:])
            eps_t = cp.tile([128, 1], F32)
            nc.vector.memset(eps_t[:], LN_EPS)
            if struct["use_bias"]:
                bias_t = cp.tile([128, HC], F32)
                nc.sync.dma_start(bias_t[:], biasb_d[:])
            if struct["use_gam"]:
                gam_t = cp.tile([128, HC], F32)
                nc.sync.dma_start(gam_t[:], gamb_d[:])
            if struct["use_bet"]:
                bet_t = cp.tile([128, HC], F32)
                nc.sync.dma_start(bet_t[:], betb_d[:])

            # ---- phase 1: xl table (all nodes), xr table (own shard) ----
            # 128-row output tiles are packed 4-wide into one SBUF tile and
            # stored with a single 256KB DMA (HWDGE descriptor-gen is the
            # phase-1 bottleneck at one DMA per 64KB tile).
            # two persistent slab buffers; the ones-row (bias input) is
            # memset once and never overwritten by the column DMAs.
            slab_a = slp.tile([65, SLAB], F16, tag="slab_a")
            slab_b = slp.tile([65, SLAB], F16, tag="slab_b")
            nc.vector.memset(slab_a[64:65, :], 1.0)
            nc.vector.memset(slab_b[64:65, :], 1.0)
            slab_ctr = [0]

            def lin_phase(src_dram, ncols, wcol0, out_dram, ev_ctr):
                nslab = (ncols + SLAB - 1) // SLAB
                for si in range(nslab):
                    c0 = si * SLAB
                    cols = min(SLAB, ncols - c0)
                    xs_t = slab_a if slab_ctr[0] % 2 == 0 else slab_b
                    slab_ctr[0] += 1
                    nc.sync.dma_start(xs_t[0:64, 0:cols], src_dram[:, c0:c0 + cols])
                    ntile = (cols + 127) // 128
                    j = 0
                    while j < ntile:
                        gsz = min(4, ntile - j)
                        evt = evp.tile([128, 4, HC], F16, tag="ev")
                        mrows_l = []
                        k = 0
                        while k < gsz:
                            mrows = min(128, cols - (j + k) * 128)
                            if mrows == 128 and k + 1 < gsz and cols - (j + k + 1) * 128 >= 128:
                                # full pair -> one 512-wide PSUM tile, one copy
                                ps = p1.tile([128, 2 * HC], F32, tag="p1")
                                for u in range(2):
                                    nc.tensor.matmul(
                                        ps[:, u * HC:(u + 1) * HC],
                                        lhsT=xs_t[0:65, (j + k + u) * 128: (j + k + u) * 128 + 128],
                                        rhs=W_t[:, wcol0:wcol0 + HC],
                                        start=True, stop=True)
                                pv = ps[:, 0:2 * HC].rearrange("p (k c) -> p k c", k=2)
                                if ev_ctr[0] % 2 == 0:
                                    nc.vector.tensor_copy(out=evt[:, k:k + 2, :], in_=pv)
                                else:
                                    nc.scalar.copy(evt[:, k:k + 2, :], pv)
                                ev_ctr[0] += 1
                                mrows_l += [128, 128]
                                k += 2
                            else:
                                ps = p1.tile([128, 2 * HC], F32, tag="p1")
                                nc.tensor.matmul(
                                    ps[0:mrows, 0:HC],
                                    lhsT=xs_t[0:65, (j + k) * 128: (j + k) * 128 + mrows],
                                    rhs=W_t[:, wcol0:wcol0 + HC],
                                    start=True, stop=True)
                                if ev_ctr[0] % 2 == 0:
                                    nc.vector.tensor_copy(out=evt[0:mrows, k, :], in_=ps[0:mrows, 0:HC])
                                else:
                                    nc.scalar.copy(evt[0:mrows, k, :], ps[0:mrows, 0:HC])
                                ev_ctr[0] += 1
                                mrows_l.append(mrows)
                                k += 1
                        r0 = c0 + j * 128
                        od, ro = out_dram(r0)
                        if all(m == 128 for m in mrows_l):
                            dst = od[ro:ro + gsz * 128, :].rearrange(
                                "(b p) c -> p b c", p=128)
                            nc.sync.dma_start(dst, evt[:, 0:gsz, :])
                        else:
                            for k2 in range(gsz):
                                nc.sync.dma_start(
                                    od[ro + k2 * 128: ro + k2 * 128 + mrows_l[k2], :],
                                    evt[0:mrows_l[k2], k2, :])
                        j += gsz

            # persistent per-window LN stats: [mean, var] per dst row; the
            # epilogue sqrt is batched over SQK windows so the ACT engine is
            # not forced to reload its function table (Prelu/Exp share one
            # table set, Sqrt does not) every window.
            SQK = 8
            stats_all = cp.tile([128, NW, 2], F32)
            nc.vector.memset(stats_all[:], 0.0)

            rep_n = int(os.environ.get("GAT_REP", "1"))
            ctr = [0]

            def xl_sel(r0):
                if r0 < HI_BASE:
                    return xl_lo, r0
                return xl_hi, r0 - HI_BASE

            nw_run = int(os.environ.get("GAT_NW", str(NW)))
            stage = int(os.environ.get("GAT_STAGE", "9"))
            run_p1 = int(os.environ.get("GAT_P1", "1"))
            rsqrt_pow = os.environ.get("GAT_RSQRT", "act") == "pow"
            TMAX = max(T_low[w] + T_high[w] for w in range(NW))
            WHC = HC + HEADS            # 260: weighted-xl columns ++ ex16

            # ---- phase 2 stages (emitted software-pipelined across windows
            # so no engine ever waits on the same window's upstream stage) ----
            st = {}

            def stage_G(w):
                """gathers: xl[src] rows (lo/hi halves), xr[dst] rows"""
                tl, th = T_low[w], T_high[w]
                T = tl + th
                off = off_w[w]
                q0 = (3 * w) % 4
                xl_t = gp.tile([128, TMAX, HC], F16, tag="xl")
                if tl:
                    nc.gpsimd.dma_gather(
                        out_ap=xl_t[:, 0:tl, :], in_ap=xl_lo[:],
                        idxs_ap=xli_t[:, 8 * off: 8 * (off + tl)],
                        num_idxs=tl * 128, num_idxs_reg=tl * 128,
                        elem_size=HC, single_packet=False, queue_num=q0)
                if th:
                    nc.gpsimd.dma_gather(
                        out_ap=xl_t[:, tl:T, :], in_ap=xl_hi[:],
                        idxs_ap=xli_t[:, 8 * (off + tl): 8 * (off + T)],
                        num_idxs=th * 128, num_idxs_reg=th * 128,
                        elem_size=HC, single_packet=False, queue_num=(q0 + 1) % 4)
                m_t = gm.tile([128, TMAX, HC], F16, tag="xr")
                nc.gpsimd.dma_gather(
                    out_ap=m_t[:, 0:T, :], in_ap=xr_dram[:],
                    idxs_ap=xri_t[:, 8 * off: 8 * (off + T)],
                    num_idxs=T * 128, num_idxs_reg=T * 128,
                    elem_size=HC, single_packet=False, queue_num=(q0 + 2) % 4)
                st[w] = {"T": T, "off": off, "xl": xl_t, "m": m_t,
                         "R": min(WIN, SH - w * WIN)}

            def stage_A(w):
                """m = xl + xr (DVE), p = prelu(m) (ACT), both in-place"""
                s = st[w]
                T, xl_t, m_t = s["T"], s["xl"], s["m"]
                nc.vector.tensor_tensor(out=m_t[:, 0:T, :], in0=xl_t[:, 0:T, :],
                                        in1=m_t[:, 0:T, :], op=AT.add)
                if stage <= 2:
                    return
                nc.scalar.activation(m_t[:, 0:T, :], m_t[:, 0:T, :],
                                     AF.Prelu, alpha=NEG_SLOPE)

            def stage_B(w):
                """u = p*att, tree-reduce -> logits, shift, exp (into w260)"""
                s = st[w]
                T, m_t = s["T"], s["m"]
                att_bc = att_t[:, None, :].to_broadcast([128, T, HC])
                nc.vector.tensor_tensor(out=m_t[:, 0:T, :], in0=m_t[:, 0:T, :],
                                        in1=att_bc, op=AT.mult)
                u_v = m_t[:, 0:T, :].rearrange("p t (h c) -> p t h c", h=HEADS)
                tr1 = wk.tile([128, TMAX, HEADS, 32], F16, tag="tr1")
                nc.vector.tensor_tensor(out=tr1[:, 0:T], in0=u_v[:, :, :, 0:32], in1=u_v[:, :, :, 32:64], op=AT.add)
                tr2 = wk.tile([128, TMAX, HEADS, 16], F16, tag="tr2")
                nc.vector.tensor_tensor(out=tr2[:, 0:T], in0=tr1[:, 0:T, :, 0:16], in1=tr1[:, 0:T, :, 16:32], op=AT.add)
                tr3 = wk.tile([128, TMAX, HEADS, 8], F16, tag="tr3")
                nc.vector.tensor_tensor(out=tr3[:, 0:T], in0=tr2[:, 0:T, :, 0:8], in1=tr2[:, 0:T, :, 8:16], op=AT.add)
                tr4 = wk.tile([128, TMAX, HEADS, 4], F16, tag="tr4")
                nc.vector.tensor_tensor(out=tr4[:, 0:T], in0=tr3[:, 0:T, :, 0:4], in1=tr3[:, 0:T, :, 4:8], op=AT.add)
                tr5 = wk.tile([128, TMAX, HEADS, 2], F16, tag="tr5")
                nc.vector.tensor_tensor(out=tr5[:, 0:T], in0=tr4[:, 0:T, :, 0:2], in1=tr4[:, 0:T, :, 2:4], op=AT.add)
                lg = wk.tile([128, TMAX, HEADS], F16, tag="lg")
                nc.vector.tensor_tensor(out=lg[:, 0:T], in0=tr5[:, 0:T, :, 0], in1=tr5[:, 0:T, :, 1], op=AT.add)
                csh_bc = csh_t[:, None, :].to_broadcast([128, T, HEADS])
                nc.vector.tensor_tensor(out=lg[:, 0:T], in0=lg[:, 0:T], in1=csh_bc, op=AT.subtract)
                if stage <= 5:
                    return
                w260 = wc.tile([128, TMAX, WHC], F16, tag="w260")
                nc.scalar.activation(w260[:, 0:T, HC:WHC], lg[:, 0:T], AF.Exp)
                lg_bc = lg[:, 0:T, :, None].to_broadcast([128, T, HEADS, OUT_C])
                nc.scalar.activation(
                    w260[:, 0:T, 0:HC].rearrange("p t (h c) -> p t h c", h=HEADS),
                    lg_bc, AF.Exp)
                s["w260"] = w260

            def stage_C(w):
                """w260 *= xl (DVE), onehot (DVE), scatter matmuls (PE)"""
                s = st[w]
                T, off, xl_t, w260 = s["T"], s["off"], s["xl"], s["w260"]
                nc.vector.tensor_tensor(out=w260[:, 0:T, 0:HC], in0=w260[:, 0:T, 0:HC],
                                        in1=xl_t[:, 0:T, :], op=AT.mult)
                oh = wc.tile([128, TMAX, 128], F16, tag="oh")
                iota_bc = iota_t[:, None, :].to_broadcast([128, T, 128])
                ds_bc = ds_t[:, off:off + T, None].to_broadcast([128, T, 128])
                nc.vector.tensor_tensor(out=oh[:, 0:T], in0=iota_bc, in1=ds_bc, op=AT.is_equal)
                if stage <= 7:
                    return
                psA = pA.tile([128, WHC], F32, tag="pA")
                for sub in range(T):
                    nc.tensor.matmul(psA[:], lhsT=oh[:, sub, :], rhs=w260[:, sub, :],
                                     start=(sub == 0), stop=(sub == T - 1))
                s["oh"] = oh
                s["psA"] = psA

            pend = {}

            def stage_D1(w):
                """alpha-normalize + LN stats; LN apply deferred to flush"""
                s = st.pop(w)
                R, psA = s["R"], s["psA"]
                rc = lnp.tile([128, HEADS], F32, tag="rc")
                nc.vector.reciprocal(out=rc[0:R, :], in_=psA[0:R, HC:WHC])
                z = zp.tile([128, HC], F32, tag="z")
                rc_bc = rc[:, :, None].to_broadcast([128, HEADS, OUT_C])
                nc.vector.tensor_tensor(
                    out=z[0:R, :].rearrange("p (h c) -> p h c", h=HEADS),
                    in0=psA[0:R, 0:HC].rearrange("p (h c) -> p h c", h=HEADS),
                    in1=rc_bc[0:R], op=AT.mult)
                if struct["use_bias"]:
                    nc.vector.tensor_tensor(out=z[0:R, :], in0=z[0:R, :], in1=bias_t[0:R, :], op=AT.add)
                st6 = lnp.tile([128, 6], F32, tag="st6")
                nc.vector.bn_stats(out=st6[0:R, :], in_=z[0:R, :])
                nc.vector.bn_aggr(out=stats_all[0:R, w, :], in_=st6[0:R, :])
                pend[w] = (z, R)

            def stage_F(ws):
                """batched sqrt over SQK windows' variances, then LN apply"""
                k = len(ws)
                w0 = ws[0]
                sd = lnp.tile([128, SQK], F32, tag="sd")
                nc.scalar.activation(sd[:, 0:k], stats_all[:, w0:w0 + k, 1],
                                     AF.Sqrt, bias=eps_t[:, :])
                rstd = lnp.tile([128, SQK], F32, tag="rstd")
                nc.vector.reciprocal(out=rstd[:, 0:k], in_=sd[:, 0:k])
                for i, w in enumerate(ws):
                    z, R = pend.pop(w)
                    yt = lnp.tile([128, HC], F32, tag="yt")
                    nc.vector.tensor_scalar(out=yt[0:R, :], in0=z[0:R, :],
                                            scalar1=stats_all[0:R, w, 0:1],
                                            scalar2=rstd[0:R, i:i + 1],
                                            op0=AT.subtract, op1=AT.mult)
                    if struct["use_gam"]:
                        nc.vector.tensor_tensor(out=yt[0:R, :], in0=yt[0:R, :], in1=gam_t[0:R, :], op=AT.mult)
                    if struct["use_bet"]:
                        nc.vector.tensor_tensor(out=yt[0:R, :], in0=yt[0:R, :], in1=bet_t[0:R, :], op=AT.add)
                    nc.vector.tensor_scalar(out=yt[0:R, :], in0=yt[0:R, :],
                                            scalar1=0.0, scalar2=None, op0=AT.max)
                    nc.sync.dma_start(y_d[w * WIN: w * WIN + R, :], yt[0:R, :])

            for _rep in range(rep_n):
              if run_p1:
                lin_phase(xT_d, N, 0, xl_sel, ctr)
                lin_phase(xTs_d, SH, HC, lambda r0: (xr_dram, r0), ctr)

              # ---- phase 2: pipelined emission G/A/B/C/D offset by one
              # window each so every engine streams back-to-back work ----
              ready = []
              for it in range(nw_run + 4):
                  if it < nw_run:
                      stage_G(it)
                  if stage >= 2 and 0 <= it - 1 < nw_run:
                      stage_A(it - 1)
                  if stage >= 4 and 0 <= it - 2 < nw_run:
                      stage_B(it - 2)
                  if stage >= 7 and 0 <= it - 3 < nw_run:
                      stage_C(it - 3)
                  if stage >= 9 and 0 <= it - 4 < nw_run:
                      stage_D1(it - 4)
                      ready.append(it - 4)
                      if len(ready) == SQK or it - 4 == nw_run - 1:
                          stage_F(ready)
                          ready = []

    nc.compile()
    return nc


_CACHE = {}


def _make_runner(nc):
    """Build a cached PJRT runner for the 8-core SPMD program (mirrors
    bass2jax.run_bass_via_pjrt, but reusable for repeat-timing)."""
    import jax
    import numpy as _np
    from jax.sharding import Mesh, PartitionSpec
    from jax.experimental.shard_map import shard_map
    import concourse.mybir as mybir
    from concourse.bass2jax import (_bass_exec_p, install_neuronx_cc_hook,
                                    partition_id_tensor)
    install_neuronx_cc_hook()

    partition_name = nc.partition_id_tensor.name if nc.partition_id_tensor else None
    in_names, out_names, out_avals, zero_outs = [], [], [], []
    for alloc in nc.m.functions[0].allocations:
        if not isinstance(alloc, mybir.MemoryLocationSet):
            continue
        name = alloc.memorylocations[0].name
        if alloc.kind == "ExternalInput":
            if name != partition_name:
                in_names.append(name)
        elif alloc.kind == "ExternalOutput":
            out_names.append(name)
            shape = tuple(alloc.tensor_shape)
            dtype = mybir.dt.np(alloc.dtype)
            out_avals.append(jax.core.ShapedArray(shape, dtype))
            zero_outs.append(_np.zeros(shape, dtype))
    n_params = len(in_names)
    n_outs = len(out_avals)
    all_names = in_names + out_names + ([partition_name] if partition_name else [])

    def _body(*args):
        operands = list(args)
        if partition_name is not None:
            operands.append(partition_id_tensor())
        return tuple(_bass_exec_p.bind(
            *operands, out_avals=tuple(out_avals), in_names=tuple(all_names),
            out_names=tuple(out_names), lowering_input_output_aliases=(),
            sim_require_finite=True, sim_require_nnan=True, nc=nc))

    devices = jax.devices()[:NCORES]
    mesh = Mesh(_np.asarray(devices), ("core",))
    sharded = jax.jit(
        shard_map(_body, mesh=mesh,
                  in_specs=(PartitionSpec("core"),) * (n_params + n_outs),
                  out_specs=(PartitionSpec("core"),) * n_outs, check_rep=False),
        keep_unused=True)

    def run(per_core, bench_iters=0):
        import time as _time
        concat_in = [
            _np.concatenate([_np.asarray(per_core[c][nm]) for c in range(NCORES)], axis=0)
            for nm in in_names]
        concat_zeros = [_np.zeros((NCORES * z.shape[0], *z.shape[1:]), z.dtype)
                        for z in zero_outs]
        dev_in = [jax.device_put(a) for a in concat_in]
        dev_z = [jax.device_put(a) for a in concat_zeros]
        out = sharded(*dev_in, *dev_z)
        jax.block_until_ready(out)
        times = []
        for _ in range(bench_iters):
            t0 = _time.perf_counter()
            out2 = sharded(*dev_in, *dev_z)
            jax.block_until_ready(out2)
            times.append(_time.perf_counter() - t0)
        res = [{nm: _np.asarray(out[i]).reshape(NCORES, *out_avals[i].shape)[c]
                for i, nm in enumerate(out_names)} for c in range(NCORES)]
        return res, times

    return run


def kernel(**inputs):
    per_core, struct = _prep(
        inputs["x"], inputs["edge_index"], inputs["W_l"], inputs["b_l"],
        inputs["W_r"], inputs["b_r"], inputs["att"], inputs["bias"],
        inputs["ln_gamma"], inputs["ln_beta"])

    key = (struct["S_total"], tuple(struct["T_low"]), tuple(struct["T_high"]),
           struct["use_bias"], struct["use_gam"], struct["use_bet"],
           os.environ.get("GAT_REP", "1"), os.environ.get("GAT_NW", ""),
           os.environ.get("GAT_P1", "1"), os.environ.get("GAT_STAGE", "9"),
           os.environ.get("GAT_RSQRT", "pow"))
    if key not in _CACHE:
        _CACHE[key] = _make_runner(_build(struct))
    run = _CACHE[key]

    bench = int(os.environ.get("GAT_BENCH", "0"))
    results, times = run(per_core, bench_iters=bench)
    out = np.concatenate([results[c]["y"] for c in range(NCORES)], axis=0)
    kernel.last_times = times
    return out.astype(np.float32)



# revision 12
# speedup vs baseline: 1.6438x; 1.6438x over previous
"""GATv2 layer (4 heads x 64ch, N=50000, E=800000) on 8 Trainium2 NeuronCores.

Strategy (dst-sharded, SPMD single NEFF):
- Host: add self-loops, shard edges by dst node (6250 dst/core), sort by dst,
  group into 128-dst windows, pack into 128-edge subtiles (padded to a common
  per-window subtile count across cores so one program serves all 8 cores).
- Phase 1 (device, replicated): xl = x @ W_l + b_l for ALL nodes -> fp16 table
  in DRAM (each core computes its own copy; avoids cross-core collectives);
  xr = x @ W_r + b_r for the core's own dst shard.
- Phase 2 (device): per window, dma_gather xl[src] / xr[dst] rows (512B fp16),
  m = xl+xr (DVE), leaky-relu via Prelu(0.2) (ACT), u = p*att (DVE),
  per-head tree reduction -> logits, shifted exp (fp16-safe), onehot(dst slot)
  built by is_equal vs iota, PE matmuls accumulate sum(ex * xl[src]) and
  sum(ex) per 128-dst window in PSUM.  Normalization is factored out of the
  per-edge path: out = num/den at window end, then +bias, LayerNorm, ReLU.
"""
import sys
import os
import numpy as np

sys.path.insert(0, '/opt/trn_rl_repo')

N = 50000
IN_C = 64
OUT_C = 64
HEADS = 4
HC = HEADS * OUT_C          # 256
E = 800000
NEG_SLOPE = 0.2
LN_EPS = 1e-5
NCORES = 8
SH = N // NCORES            # 6250 dst nodes per core
WIN = 128
NW = (SH + WIN - 1) // WIN  # 49 windows (last window 106 dst)
HI_BASE = 32768             # int16 gather index split point
SLAB = 2048                 # phase-1 column slab


def _pack_idx(v):
    """Flat idx vector (len % 16 == 0) -> [128, len/16] int16 (idx k at
    (k%16, k//16), replicated across the 8 gpsimd cores' 16-partition stripes)."""
    a = v.reshape(-1, 16).T
    return np.tile(a, (8, 1)).astype(np.int16)


def _prep(x, edge_index, W_l, b_l, W_r, b_r, att, bias, ln_gamma, ln_beta):
    src = np.asarray(edge_index[0], dtype=np.int64)
    dst = np.asarray(edge_index[1], dtype=np.int64)
    loops = np.arange(N, dtype=np.int64)
    src = np.concatenate([src, loops])
    dst = np.concatenate([dst, loops])

    core = dst // SH
    dstl = dst % SH
    win = dstl // WIN
    half = (src >= HI_BASE).astype(np.int64)

    order = np.lexsort((dstl, half, win, core))
    src = src[order]; dst = dst[order]
    core = core[order]; dstl = dstl[order]; win = win[order]; half = half[order]

    # counts[core, win, half]
    key = (core * NW + win) * 2 + half
    cnt = np.bincount(key, minlength=NCORES * NW * 2).reshape(NCORES, NW, 2)
    subs = -(-cnt // 128)                      # ceil
    T_lh = subs.max(axis=0)                    # [NW, 2] shared subtile counts
    T_low = T_lh[:, 0]; T_high = T_lh[:, 1]
    T_w = T_low + T_high
    S_total = int(T_w.sum())
    off_w = np.zeros(NW, dtype=np.int64)
    off_w[1:] = np.cumsum(T_w)[:-1]

    # slot base for each (core, win, half) group in the flat padded stream
    grp_off = np.zeros((NCORES, NW, 2), dtype=np.int64)
    for c in range(NCORES):
        for w in range(NW):
            grp_off[c, w, 0] = off_w[w] * 128
            grp_off[c, w, 1] = (off_w[w] + T_low[w]) * 128

    NSLOT = S_total * 128
    # within-group rank of each edge (edges are sorted by (core,win,half))
    grp_start = np.zeros(NCORES * NW * 2, dtype=np.int64)
    grp_start[1:] = np.cumsum(cnt.reshape(-1))[:-1]
    rank = np.arange(len(src)) - grp_start[key]
    slot = grp_off[core, win, half] + rank

    xl_idx = np.zeros((NCORES, NSLOT), dtype=np.int16)
    xr_idx = np.zeros((NCORES, NSLOT), dtype=np.int16)
    ds_val = np.full((NCORES, NSLOT), 254.0, dtype=np.float16)
    xl_val = np.where(half == 1, src - HI_BASE, src).astype(np.int16)
    for c in range(NCORES):
        m = core == c
        xl_idx[c, slot[m]] = xl_val[m]
        xr_idx[c, slot[m]] = dstl[m].astype(np.int16)
        ds_val[c, slot[m]] = (dstl[m] - win[m] * WIN).astype(np.float16)

    # per-head exp shift from a sample of edges (keeps exp in fp16 range)
    rs = np.random.RandomState(1234)
    samp = rs.randint(0, len(src), min(32768, len(src)))
    xs = np.asarray(x, dtype=np.float32)
    Wl = np.asarray(W_l, np.float32); Wr = np.asarray(W_r, np.float32)
    bl = np.asarray(b_l, np.float32); br = np.asarray(b_r, np.float32)
    attf = np.asarray(att, np.float32).reshape(HEADS, OUT_C)
    ms = (xs[src[samp]] @ Wl + bl) + (xs[dst[samp]] @ Wr + br)
    ls = np.where(ms > 0, ms, NEG_SLOPE * ms).reshape(-1, HEADS, OUT_C)
    lg = np.einsum('ehc,hc->eh', ls, attf)
    c_shift = (lg.max(axis=0) - 6.0).astype(np.float32)

    # device-side constant tensors
    xT = np.ascontiguousarray(xs.T).astype(np.float16)              # [64, N]
    W_aug = np.zeros((65, 2 * HC), dtype=np.float16)
    W_aug[:64, :HC] = Wl; W_aug[64, :HC] = bl
    W_aug[:64, HC:] = Wr; W_aug[64, HC:] = br
    att_b = np.broadcast_to(attf.reshape(-1).astype(np.float16), (128, HC)).copy()
    iota = np.broadcast_to(np.arange(128, dtype=np.float16), (128, 128)).copy()
    c_b = np.broadcast_to(c_shift, (128, HEADS)).copy()

    biasf = np.asarray(bias, np.float32)
    gam = np.asarray(ln_gamma, np.float32)
    bet = np.asarray(ln_beta, np.float32)
    use_bias = bool(np.any(biasf != 0.0))
    use_gam = bool(np.any(gam != 1.0))
    use_bet = bool(np.any(bet != 0.0))

    per_core = []
    for c in range(NCORES):
        per_core.append({
            "xT": xT,
            "xTs": np.ascontiguousarray(xT[:, c * SH:(c + 1) * SH]),
            "W": W_aug,
            "attb": att_b,
            "iota": iota,
            "cshift": c_b.astype(np.float16),
            "xli": _pack_idx(xl_idx[c]),
            "xri": _pack_idx(xr_idx[c]),
            "dslot": np.ascontiguousarray(ds_val[c].reshape(-1, 128).T).astype(np.float16),
            "biasb": np.broadcast_to(biasf, (128, HC)).astype(np.float32).copy(),
            "gamb": np.broadcast_to(gam, (128, HC)).astype(np.float32).copy(),
            "betb": np.broadcast_to(bet, (128, HC)).astype(np.float32).copy(),
        })
    struct = {
        "T_low": T_low.tolist(), "T_high": T_high.tolist(),
        "off_w": off_w.tolist(), "S_total": S_total,
        "use_bias": use_bias, "use_gam": use_gam, "use_bet": use_bet,
    }
    return per_core, struct


def _build(struct):
    import concourse.bacc as bacc
    import concourse.mybir as mybir
    import concourse.tile as tile

    F16 = mybir.dt.float16
    F32 = mybir.dt.float32
    I16 = mybir.dt.int16
    AT = mybir.AluOpType
    AF = mybir.ActivationFunctionType

    T_low = struct["T_low"]; T_high = struct["T_high"]
    off_w = struct["off_w"]; S = struct["S_total"]

    nc = bacc.Bacc("TRN2", num_devices=NCORES, num_swdge_queues=4)

    xT_d = nc.dram_tensor("xT", [64, N], F16, kind="ExternalInput")
    xTs_d = nc.dram_tensor("xTs", [64, SH], F16, kind="ExternalInput")
    W_d = nc.dram_tensor("W", [65, 2 * HC], F16, kind="ExternalInput")
    attb_d = nc.dram_tensor("attb", [128, HC], F16, kind="ExternalInput")
    iota_d = nc.dram_tensor("iota", [128, 128], F16, kind="ExternalInput")
    csh_d = nc.dram_tensor("cshift", [128, HEADS], F16, kind="ExternalInput")
    xli_d = nc.dram_tensor("xli", [128, 8 * S], I16, kind="ExternalInput")
    xri_d = nc.dram_tensor("xri", [128, 8 * S], I16, kind="ExternalInput")
    ds_d = nc.dram_tensor("dslot", [128, S], F16, kind="ExternalInput")
    biasb_d = nc.dram_tensor("biasb", [128, HC], F32, kind="ExternalInput")
    gamb_d = nc.dram_tensor("gamb", [128, HC], F32, kind="ExternalInput")
    betb_d = nc.dram_tensor("betb", [128, HC], F32, kind="ExternalInput")
    y_d = nc.dram_tensor("y", [SH, HC], F16, kind="ExternalOutput")

    xl_lo = nc.dram_tensor("xl_lo", [HI_BASE, HC], F16, kind="Internal")
    xl_hi = nc.dram_tensor("xl_hi", [N - HI_BASE, HC], F16, kind="Internal")
    xr_dram = nc.dram_tensor("xr_tab", [SH, HC], F16, kind="Internal")

    with tile.TileContext(nc) as tc:
        with tc.tile_pool(name="const", bufs=1) as cp, \
             tc.tile_pool(name="slab", bufs=2) as slp, \
             tc.tile_pool(name="ev", bufs=3) as evp, \
             tc.tile_pool(name="g", bufs=4) as gp, \
             tc.tile_pool(name="gm", bufs=3) as gm, \
             tc.tile_pool(name="wk", bufs=2) as wk, \
             tc.tile_pool(name="wc", bufs=3) as wc, \
             tc.tile_pool(name="ln", bufs=3) as lnp, \
             tc.tile_pool(name="zr", bufs=12) as zp, \
             tc.tile_pool(name="p1", bufs=4, space="PSUM") as p1, \
             tc.tile_pool(name="pA", bufs=3, space="PSUM") as pA:

            # ---- constants ----
            W_t = cp.tile([65, 2 * HC], F16)
            nc.sync.dma_start(W_t[:], W_d[:])
            att_t = cp.tile([128, HC], F16)
            nc.sync.dma_start(att_t[:], attb_d[:])
            iota_t = cp.tile([128, 128], F16)
            nc.sync.dma_start(iota_t[:], iota_d[:])
            csh_t = cp.tile([128, HEADS], F16)
            nc.sync.dma_start(csh_t[:], csh_d[:])
            xli_t = cp.tile([128, 8 * S], I16)
            nc.sync.dma_start(xli_t[:], xli_d[:])
            xri_t = cp.tile([128, 8 * S], I16)
            nc.sync.dma_start(xri_t[:], xri_d[:])
            ds_t = cp.tile([128, S], F16)
            nc.sync.dma_start(ds_t[:], ds_d[:])
            eps_t = cp.tile([128, 1], F32)
            nc.vector.memset(eps_t[:], LN_EPS)
            if struct["use_bias"]:
                bias_t = cp.tile([128, HC], F32)
                nc.sync.dma_start(bias_t[:], biasb_d[:])
            if struct["use_gam"]:
                gam_t = cp.tile([128, HC], F32)
                nc.sync.dma_start(gam_t[:], gamb_d[:])
            if struct["use_bet"]:
                bet_t = cp.tile([128, HC], F32)
                nc.sync.dma_start(bet_t[:], betb_d[:])

            # ---- phase 1: xl table (all nodes), xr table (own shard) ----
            # 128-row output tiles are packed 4-wide into one SBUF tile and
            # stored with a single 256KB DMA (HWDGE descriptor-gen is the
            # phase-1 bottleneck at one DMA per 64KB tile).
            # two persistent slab buffers; the ones-row (bias input) is
            # memset once and never overwritten by the column DMAs.
            slab_a = slp.tile([65, SLAB], F16, tag="slab_a")
            slab_b = slp.tile([65, SLAB], F16, tag="slab_b")
            nc.vector.memset(slab_a[64:65, :], 1.0)
            nc.vector.memset(slab_b[64:65, :], 1.0)
            slab_ctr = [0]

            def lin_phase(src_dram, ncols, wcol0, out_dram, ev_ctr):
                nslab = (ncols + SLAB - 1) // SLAB
                for si in range(nslab):
                    c0 = si * SLAB
                    cols = min(SLAB, ncols - c0)
                    xs_t = slab_a if slab_ctr[0] % 2 == 0 else slab_b
                    slab_ctr[0] += 1
                    nc.sync.dma_start(xs_t[0:64, 0:cols], src_dram[:, c0:c0 + cols])
                    ntile = (cols + 127) // 128
                    j = 0
                    while j < ntile:
                        gsz = min(4, ntile - j)
                        evt = evp.tile([128, 4, HC], F16, tag="ev")
                        mrows_l = []
                        k = 0
                        while k < gsz:
                            mrows = min(128, cols - (j + k) * 128)
                            if mrows == 128 and k + 1 < gsz and cols - (j + k + 1) * 128 >= 128:
                                # full pair -> one 512-wide PSUM tile, one copy
                                ps = p1.tile([128, 2 * HC], F32, tag="p1")
                                for u in range(2):
                                    nc.tensor.matmul(
                                        ps[:, u * HC:(u + 1) * HC],
                                        lhsT=xs_t[0:65, (j + k + u) * 128: (j + k + u) * 128 + 128],
                                        rhs=W_t[:, wcol0:wcol0 + HC],
                                        start=True, stop=True)
                                pv = ps[:, 0:2 * HC].rearrange("p (k c) -> p k c", k=2)
                                if ev_ctr[0] % 2 == 0:
                                    nc.vector.tensor_copy(out=evt[:, k:k + 2, :], in_=pv)
                                else:
                                    nc.scalar.copy(evt[:, k:k + 2, :], pv)
                                ev_ctr[0] += 1
                                mrows_l += [128, 128]
                                k += 2
                            else:
                                ps = p1.tile([128, 2 * HC], F32, tag="p1")
                                nc.tensor.matmul(
                                    ps[0:mrows, 0:HC],
                                    lhsT=xs_t[0:65, (j + k) * 128: (j + k) * 128 + mrows],
                                    rhs=W_t[:, wcol0:wcol0 + HC],
                                    start=True, stop=True)
                                if ev_ctr[0] % 2 == 0:
                                    nc.vector.tensor_copy(out=evt[0:mrows, k, :], in_=ps[0:mrows, 0:HC])
                                else:
                                    nc.scalar.copy(evt[0:mrows, k, :], ps[0:mrows, 0:HC])
                                ev_ctr[0] += 1
                                mrows_l.append(mrows)
                                k += 1
                        r0 = c0 + j * 128
                        od, ro = out_dram(r0)
                        if all(m == 128 for m in mrows_l):
                            dst = od[ro:ro + gsz * 128, :].rearrange(
                                "(b p) c -> p b c", p=128)
                            nc.sync.dma_start(dst, evt[:, 0:gsz, :])
                        else:
                            for k2 in range(gsz):
                                nc.sync.dma_start(
                                    od[ro + k2 * 128: ro + k2 * 128 + mrows_l[k2], :],
                                    evt[0:mrows_l[k2], k2, :])
                        j += gsz

            # persistent per-window LN stats: [mean, var] per dst row; the
            # epilogue sqrt is batched over SQK windows so the ACT engine is
            # not forced to reload its function table (Prelu/Exp share one
            # table set, Sqrt does not) every window.
            SQK = 8
            stats_all = cp.tile([128, NW, 2], F32)
            nc.vector.memset(stats_all[:], 0.0)

            rep_n = int(os.environ.get("GAT_REP", "1"))
            ctr = [0]

            def xl_sel(r0):
                if r0 < HI_BASE:
                    return xl_lo, r0
                return xl_hi, r0 - HI_BASE

            nw_run = int(os.environ.get("GAT_NW", str(NW)))
            stage = int(os.environ.get("GAT_STAGE", "9"))
            run_p1 = int(os.environ.get("GAT_P1", "1"))
            rsqrt_pow = os.environ.get("GAT_RSQRT", "act") == "pow"
            TMAX = max(T_low[w] + T_high[w] for w in range(NW))
            WHC = HC + HEADS            # 260: weighted-xl columns ++ ex16

            # ---- phase 2 stages (emitted software-pipelined across windows
            # so no engine ever waits on the same window's upstream stage) ----
            st = {}

            def stage_G(w):
                """gathers: xl[src] rows (lo/hi halves), xr[dst] rows"""
                tl, th = T_low[w], T_high[w]
                T = tl + th
                off = off_w[w]
                q0 = (3 * w) % 4
                xl_t = gp.tile([128, TMAX, HC], F16, tag="xl")
                if tl:
                    nc.gpsimd.dma_gather(
                        out_ap=xl_t[:, 0:tl, :], in_ap=xl_lo[:],
                        idxs_ap=xli_t[:, 8 * off: 8 * (off + tl)],
                        num_idxs=tl * 128, num_idxs_reg=tl * 128,
                        elem_size=HC, single_packet=False, queue_num=q0)
                if th:
                    nc.gpsimd.dma_gather(
                        out_ap=xl_t[:, tl:T, :], in_ap=xl_hi[:],
                        idxs_ap=xli_t[:, 8 * (off + tl): 8 * (off + T)],
                        num_idxs=th * 128, num_idxs_reg=th * 128,
                        elem_size=HC, single_packet=False, queue_num=(q0 + 1) % 4)
                m_t = gm.tile([128, TMAX, HC], F16, tag="xr")
                nc.gpsimd.dma_gather(
                    out_ap=m_t[:, 0:T, :], in_ap=xr_dram[:],
                    idxs_ap=xri_t[:, 8 * off: 8 * (off + T)],
                    num_idxs=T * 128, num_idxs_reg=T * 128,
                    elem_size=HC, single_packet=False, queue_num=(q0 + 2) % 4)
                st[w] = {"T": T, "off": off, "xl": xl_t, "m": m_t,
                         "R": min(WIN, SH - w * WIN)}

            def stage_A(w):
                """m = xl + xr (DVE), p = prelu(m) (ACT), both in-place"""
                s = st[w]
                T, xl_t, m_t = s["T"], s["xl"], s["m"]
                nc.vector.tensor_tensor(out=m_t[:, 0:T, :], in0=xl_t[:, 0:T, :],
                                        in1=m_t[:, 0:T, :], op=AT.add)
                if stage <= 2:
                    return
                nc.scalar.activation(m_t[:, 0:T, :], m_t[:, 0:T, :],
                                     AF.Prelu, alpha=NEG_SLOPE)

            def stage_B(w):
                """u = p*att, tree-reduce -> logits, shift, exp (into w260)"""
                s = st[w]
                T, m_t = s["T"], s["m"]
                att_bc = att_t[:, None, :].to_broadcast([128, T, HC])
                nc.vector.tensor_tensor(out=m_t[:, 0:T, :], in0=m_t[:, 0:T, :],
                                        in1=att_bc, op=AT.mult)
                u_v = m_t[:, 0:T, :].rearrange("p t (h c) -> p t h c", h=HEADS)
                tr1 = wk.tile([128, TMAX, HEADS, 32], F16, tag="tr1")
                nc.vector.tensor_tensor(out=tr1[:, 0:T], in0=u_v[:, :, :, 0:32], in1=u_v[:, :, :, 32:64], op=AT.add)
                tr2 = wk.tile([128, TMAX, HEADS, 16], F16, tag="tr2")
                nc.vector.tensor_tensor(out=tr2[:, 0:T], in0=tr1[:, 0:T, :, 0:16], in1=tr1[:, 0:T, :, 16:32], op=AT.add)
                tr3 = wk.tile([128, TMAX, HEADS, 8], F16, tag="tr3")
                nc.vector.tensor_tensor(out=tr3[:, 0:T], in0=tr2[:, 0:T, :, 0:8], in1=tr2[:, 0:T, :, 8:16], op=AT.add)
                tr4 = wk.tile([128, TMAX, HEADS, 4], F16, tag="tr4")
                nc.vector.tensor_tensor(out=tr4[:, 0:T], in0=tr3[:, 0:T, :, 0:4], in1=tr3[:, 0:T, :, 4:8], op=AT.add)
                tr5 = wk.tile([128, TMAX, HEADS, 2], F16, tag="tr5")
                nc.vector.tensor_tensor(out=tr5[:, 0:T], in0=tr4[:, 0:T, :, 0:2], in1=tr4[:, 0:T, :, 2:4], op=AT.add)
                lg = wk.tile([128, TMAX, HEADS], F16, tag="lg")
                nc.vector.tensor_tensor(out=lg[:, 0:T], in0=tr5[:, 0:T, :, 0], in1=tr5[:, 0:T, :, 1], op=AT.add)
                csh_bc = csh_t[:, None, :].to_broadcast([128, T, HEADS])
                nc.vector.tensor_tensor(out=lg[:, 0:T], in0=lg[:, 0:T], in1=csh_bc, op=AT.subtract)
                if stage <= 5:
                    return
                w260 = wc.tile([128, TMAX, WHC], F16, tag="w260")
                nc.scalar.activation(w260[:, 0:T, HC:WHC], lg[:, 0:T], AF.Exp)
                lg_bc = lg[:, 0:T, :, None].to_broadcast([128, T, HEADS, OUT_C])
                nc.scalar.activation(
                    w260[:, 0:T, 0:HC].rearrange("p t (h c) -> p t h c", h=HEADS),
                    lg_bc, AF.Exp)
                s["w260"] = w260

            def stage_C(w):
                """w260 *= xl (DVE), onehot (DVE), scatter matmuls (PE)"""
                s = st[w]
                T, off, xl_t, w260 = s["T"], s["off"], s["xl"], s["w260"]
                nc.vector.tensor_tensor(out=w260[:, 0:T, 0:HC], in0=w260[:, 0:T, 0:HC],
                                        in1=xl_t[:, 0:T, :], op=AT.mult)
                oh = wc.tile([128, TMAX, 128], F16, tag="oh")
                iota_bc = iota_t[:, None, :].to_broadcast([128, T, 128])
                ds_bc = ds_t[:, off:off + T, None].to_broadcast([128, T, 128])
                nc.vector.tensor_tensor(out=oh[:, 0:T], in0=iota_bc, in1=ds_bc, op=AT.is_equal)
                if stage <= 7:
                    return
                psA = pA.tile([128, WHC], F32, tag="pA")
                for sub in range(T):
                    nc.tensor.matmul(psA[:], lhsT=oh[:, sub, :], rhs=w260[:, sub, :],
                                     start=(sub == 0), stop=(sub == T - 1))
                s["oh"] = oh
                s["psA"] = psA

            pend = {}

            def stage_D1(w):
                """alpha-normalize + LN stats; LN apply deferred to flush"""
                s = st.pop(w)
                R, psA = s["R"], s["psA"]
                rc = lnp.tile([128, HEADS], F32, tag="rc")
                nc.vector.reciprocal(out=rc[0:R, :], in_=psA[0:R, HC:WHC])
                z = zp.tile([128, HC], F16, tag="z")
                rc_bc = rc[:, :, None].to_broadcast([128, HEADS, OUT_C])
                nc.vector.tensor_tensor(
                    out=z[0:R, :].rearrange("p (h c) -> p h c", h=HEADS),
                    in0=psA[0:R, 0:HC].rearrange("p (h c) -> p h c", h=HEADS),
                    in1=rc_bc[0:R], op=AT.mult)
                if struct["use_bias"]:
                    nc.vector.tensor_tensor(out=z[0:R, :], in0=z[0:R, :], in1=bias_t[0:R, :], op=AT.add)
                st6 = lnp.tile([128, 6], F32, tag="st6")
                nc.vector.bn_stats(out=st6[0:R, :], in_=z[0:R, :])
                nc.vector.bn_aggr(out=stats_all[0:R, w, :], in_=st6[0:R, :])
                pend[w] = (z, R)

            def stage_F(ws):
                """batched sqrt over SQK windows' variances, then LN apply"""
                k = len(ws)
                w0 = ws[0]
                sd = lnp.tile([128, SQK], F32, tag="sd")
                nc.scalar.activation(sd[:, 0:k], stats_all[:, w0:w0 + k, 1],
                                     AF.Sqrt, bias=eps_t[:, :])
                rstd = lnp.tile([128, SQK], F32, tag="rstd")
                nc.vector.reciprocal(out=rstd[:, 0:k], in_=sd[:, 0:k])
                for i, w in enumerate(ws):
                    z, R = pend.pop(w)
                    yt = lnp.tile([128, HC], F16, tag="yt")
                    nc.vector.tensor_scalar(out=yt[0:R, :], in0=z[0:R, :],
                                            scalar1=stats_all[0:R, w, 0:1],
                                            scalar2=rstd[0:R, i:i + 1],
                                            op0=AT.subtract, op1=AT.mult)
                    if struct["use_gam"]:
                        nc.vector.tensor_tensor(out=yt[0:R, :], in0=yt[0:R, :], in1=gam_t[0:R, :], op=AT.mult)
                    if struct["use_bet"]:
                        nc.vector.tensor_tensor(out=yt[0:R, :], in0=yt[0:R, :], in1=bet_t[0:R, :], op=AT.add)
                    nc.vector.tensor_scalar(out=yt[0:R, :], in0=yt[0:R, :],
                                            scalar1=0.0, scalar2=None, op0=AT.max)
                    nc.sync.dma_start(y_d[w * WIN: w * WIN + R, :], yt[0:R, :])

            for _rep in range(rep_n):
              if run_p1:
                lin_phase(xT_d, N, 0, xl_sel, ctr)
                lin_phase(xTs_d, SH, HC, lambda r0: (xr_dram, r0), ctr)

              # ---- phase 2: pipelined emission G/A/B/C/D offset by one
              # window each so every engine streams back-to-back work ----
              ready = []
              for it in range(nw_run + 4):
                  if it < nw_run:
                      stage_G(it)
                  if stage >= 2 and 0 <= it - 1 < nw_run:
                      stage_A(it - 1)
                  if stage >= 4 and 0 <= it - 2 < nw_run:
                      stage_B(it - 2)
                  if stage >= 7 and 0 <= it - 3 < nw_run:
                      stage_C(it - 3)
                  if stage >= 9 and 0 <= it - 4 < nw_run:
                      stage_D1(it - 4)
                      ready.append(it - 4)
                      if len(ready) == SQK or it - 4 == nw_run - 1:
                          stage_F(ready)
                          ready = []

    nc.compile()
    return nc


_CACHE = {}


def _make_runner(nc):
    """Build a cached PJRT runner for the 8-core SPMD program (mirrors
    bass2jax.run_bass_via_pjrt, but reusable for repeat-timing)."""
    import jax
    import numpy as _np
    from jax.sharding import Mesh, PartitionSpec
    from jax.experimental.shard_map import shard_map
    import concourse.mybir as mybir
    from concourse.bass2jax import (_bass_exec_p, install_neuronx_cc_hook,
                                    partition_id_tensor)
    install_neuronx_cc_hook()

    partition_name = nc.partition_id_tensor.name if nc.partition_id_tensor else None
    in_names, out_names, out_avals, zero_outs = [], [], [], []
    for alloc in nc.m.functions[0].allocations:
        if not isinstance(alloc, mybir.MemoryLocationSet):
            continue
        name = alloc.memorylocations[0].name
        if alloc.kind == "ExternalInput":
            if name != partition_name:
                in_names.append(name)
        elif alloc.kind == "ExternalOutput":
            out_names.append(name)
            shape = tuple(alloc.tensor_shape)
            dtype = mybir.dt.np(alloc.dtype)
            out_avals.append(jax.core.ShapedArray(shape, dtype))
            zero_outs.append(_np.zeros(shape, dtype))
    n_params = len(in_names)
    n_outs = len(out_avals)
    all_names = in_names + out_names + ([partition_name] if partition_name else [])

    def _body(*args):
        operands = list(args)
        if partition_name is not None:
            operands.append(partition_id_tensor())
        return tuple(_bass_exec_p.bind(
            *operands, out_avals=tuple(out_avals), in_names=tuple(all_names),
            out_names=tuple(out_names), lowering_input_output_aliases=(),
            sim_require_finite=True, sim_require_nnan=True, nc=nc))

    devices = jax.devices()[:NCORES]
    mesh = Mesh(_np.asarray(devices), ("core",))
    sharded = jax.jit(
        shard_map(_body, mesh=mesh,
                  in_specs=(PartitionSpec("core"),) * (n_params + n_outs),
                  out_specs=(PartitionSpec("core"),) * n_outs, check_rep=False),
        keep_unused=True)

    def run(per_core, bench_iters=0):
        import time as _time
        concat_in = [
            _np.concatenate([_np.asarray(per_core[c][nm]) for c in range(NCORES)], axis=0)
            for nm in in_names]
        concat_zeros = [_np.zeros((NCORES * z.shape[0], *z.shape[1:]), z.dtype)
                        for z in zero_outs]
        dev_in = [jax.device_put(a) for a in concat_in]
        dev_z = [jax.device_put(a) for a in concat_zeros]
        out = sharded(*dev_in, *dev_z)
        jax.block_until_ready(out)
        times = []
        for _ in range(bench_iters):
            t0 = _time.perf_counter()
            out2 = sharded(*dev_in, *dev_z)
            jax.block_until_ready(out2)
            times.append(_time.perf_counter() - t0)
        res = [{nm: _np.asarray(out[i]).reshape(NCORES, *out_avals[i].shape)[c]
                for i, nm in enumerate(out_names)} for c in range(NCORES)]
        return res, times

    return run


def kernel(**inputs):
    per_core, struct = _prep(
        inputs["x"], inputs["edge_index"], inputs["W_l"], inputs["b_l"],
        inputs["W_r"], inputs["b_r"], inputs["att"], inputs["bias"],
        inputs["ln_gamma"], inputs["ln_beta"])

    key = (struct["S_total"], tuple(struct["T_low"]), tuple(struct["T_high"]),
           struct["use_bias"], struct["use_gam"], struct["use_bet"],
           os.environ.get("GAT_REP", "1"), os.environ.get("GAT_NW", ""),
           os.environ.get("GAT_P1", "1"), os.environ.get("GAT_STAGE", "9"),
           os.environ.get("GAT_RSQRT", "pow"))
    if key not in _CACHE:
        _CACHE[key] = _make_runner(_build(struct))
    run = _CACHE[key]

    bench = int(os.environ.get("GAT_BENCH", "0"))
    results, times = run(per_core, bench_iters=bench)
    out = np.concatenate([results[c]["y"] for c in range(NCORES)], axis=0)
    kernel.last_times = times
    return out.astype(np.float32)

